# revision 20
# baseline (speedup 1.0000x reference)
"""MoE (top-2 of 8 experts, SwiGLU) Trainium2 kernel, expert-parallel over 8 cores.

Contract: kernel(**inputs) takes the FULL unsharded inputs
  x [2,2048,1024] f32, gate_w [8,1024] f32,
  w1 [8,2048,1024] f32, w2 [8,1024,2048] f32, w3 [8,2048,1024] f32
and returns the FULL output [2,2048,1024] f32.

Strategy (expert-parallel, per the hint "replicate the gate and all-to-all the
token dispatch"): routing (gate softmax + top-2) is computed on host; tokens
are dispatched (gathered) per expert; core e runs the SwiGLU FFN of expert e
over its ~N*TOPK/E assigned tokens (padded to capacity C), pre-scaled by the
combine weight; the host scatter-adds the two expert contributions per token.

Device kernel (per core, feature-major layout so no on-device transposes):
  h1T = w1 @ xg^T   [H, C]
  h3T = w3 @ xg^T   [H, C]
  aT  = silu(h1T) * h3T      (ACT Silu + DVE mul, PSUM->SBUF)
  yT  = (w2 @ aT) * combine  [D, C]  (DVE mul on PSUM eviction)

Perf-critical choices (from perfetto traces of earlier revisions):
- bf16 operands: same 1 col/cycle PE rate as fp32r, half the HBM bytes, FWL
  weight loads; PSUM stays fp32, combine-scale and yT stay fp32 (~4e-3 rel
  err, inside the 2e-2 gate).
- All streamed tensors are PRE-TILED on host so every DMA moves >=2KB
  contiguous per partition (one descriptor per partition): 256B-line column
  slices of row-major weights ran the early DMA window at ~50% efficiency
  and starved the PE for its first ~20us.
- Warm-up dummy matmuls keep the PE busy from engine-boot so the HAM clock
  gate un-throttles (1.2->2.4 GHz after ~3.4us continuous activity) while
  the first input DMAs land.
- Critical first loads are split across the two HWDGE rings (xg chunk 0 on
  sync, h0/h1 weight tiles on scalar); the gpsimd weight prefetch and the
  combine-scale load are held out of that window with explicit deps.
"""

import math
import sys

import numpy as np

for _p in ("/opt/trn_rl_repo", "/opt/pypackages"):
    if _p not in sys.path:
        sys.path.append(_p)

import ml_dtypes  # noqa: E402

import concourse.bass as bass  # noqa: E402
import concourse.tile as tile  # noqa: E402
from concourse import bacc, mybir  # noqa: E402
from concourse.bass_utils import run_bass_kernel_spmd  # noqa: E402
from concourse.tile import add_dep_helper  # noqa: E402
from concourse.vector_clock import ScopedClock  # noqa: E402


class _FastExitTileContext(tile.TileContext):
    """TileContext whose exit keeps only the DMA drain (sem-gated on the
    full vector clock, so every queue's completion — including the final
    output-store receipts — is awaited before the kernel ends) and drops
    the all-engine barriers + semaphore clear. The runtime re-arms
    semaphores per execution; test.py validates this by running the kernel
    twice in one process and checking both outputs."""

    def _drain_and_barrier(self, tick_clock, wait_clock):
        drain_inst = self.nc.sync.drain()
        wait_clock.add_sem_waits(
            drain_inst.ins, ScopedClock({None: tick_clock.global_clock})
        )
        popped = self.nc._tile_sem_poison_stack.pop()
        assert popped is self._sem_poison

B, T, D, H, E, TOPK = 2, 2048, 1024, 2048, 8, 2
N = B * T
P = 128
KD = D // P   # 8  k-tiles over D
KH = H // P   # 16 k-tiles over H
HB = H // P   # 16 h blocks of 128 (M dim, stage A)
DB = D // P   # 8  d blocks of 128 (M dim, stage B)

F32 = mybir.dt.float32
BF16 = mybir.dt.bfloat16
NPBF16 = ml_dtypes.bfloat16

# warm-up matmuls (256-col) issued before the first real matmul
NDUM = 30

# set by test.py to capture an NTFF profile; kernel() stores results here
TRACE = False
TRACE_ALL_CORES = False
LAST_RESULTS = None

_program_cache = {}

# CoreSim doesn't implement Silu; simcheck.py overrides this to Sigmoid.
_ACT_FUNC = mybir.ActivationFunctionType.Silu


# Max tokens per expert handled on host when the count barely exceeds a
# 512 multiple (capacity-factor overflow).
OVERFLOW_MAX = 64


def _chunk_plan(cmax: int) -> list[int]:
    """Token-chunk sizes for the device capacity: each <=512 (PSUM bank),
    as equal as possible, 32-aligned, minimal total padding. If cmax is
    within OVERFLOW_MAX above a 512 multiple, use full 512 chunks and let
    the caller route the overflow tokens to the host FFN."""
    if cmax >= 512 and cmax - (cmax // 512) * 512 <= OVERFLOW_MAX:
        m = cmax // 512
        if m >= 2:
            # 256-col head chunk: the first matmul group gates on half the
            # xg bytes (earlier start); 256-col tail chunk: the final
            # eviction + store trail a half-size group (shorter tail).
            # bf16 matmuls run full rate at any free size.
            return [256] + [512] * (m - 1) + [256]
        return [512] * m
    n = max(1, math.ceil(cmax / 512))
    chunks = []
    rem = cmax
    for i in range(n):
        s = math.ceil(rem / (n - i) / 32) * 32
        s = min(max(s, 256), 512)
        chunks.append(s)
        rem -= s
    return chunks


def _host_ffn(x_rows, w1e, w2e, w3e, wts):
    """Exact host-side SwiGLU FFN for capacity-overflow tokens (<=64/expert)."""
    h1 = x_rows @ w1e.T
    h3 = x_rows @ w3e.T
    a = h1 / (1.0 + np.exp(-h1)) * h3
    return (a @ w2e.T) * wts[:, None]


def _pretile(w, nb):
    """[M, K] row-major weight -> [nb, P(p in K k-tile), nk*P] where block b,
    partition p, flat (k, c) reads w[b*P + c, k*P + p]: per-partition lines
    are nk*P contiguous elements (2KB bf16) in DRAM."""
    m, kdim = w.shape
    nk = kdim // P
    assert m == nb * P
    return np.ascontiguousarray(
        w.reshape(nb, P, nk, P).transpose(0, 3, 2, 1).reshape(nb, P, nk * P)
    ).astype(NPBF16)


def _build_program(chunks: list[int]):
    """Bass program for one core: expert FFN over C = sum(chunks) tokens."""
    C = sum(chunks)
    offs = [sum(chunks[:i]) for i in range(len(chunks))]
    nt = len(chunks)

    nc = bacc.Bacc(
        "TRN2", target_bir_lowering=False, debug=False,
        enable_asserts=False, num_devices=8,
    )
    # xg is chunk-blocked flat: partition p holds, per chunk t, KD runs of
    # chunk_t columns (contiguous per chunk both in DRAM and SBUF).
    xgT_d = nc.dram_tensor("xgT", [P, KD * C], BF16, kind="ExternalInput").ap()
    # w1 and w3 h-blocks interleaved so one dma_start loads both (the ~0.7us
    # per-dma_start dispatch on the issuing sequencer is what delays the
    # first matmuls, not bytes)
    w13T_d = nc.dram_tensor("w13T", [HB, P, 2 * KD * P], BF16,
                            kind="ExternalInput").ap()
    w2T_d = nc.dram_tensor("w2T", [DB, P, KH * P], BF16, kind="ExternalInput").ap()
    scl_d = nc.dram_tensor("scale_b", [P, C], F32, kind="ExternalInput").ap()
    yT_d = nc.dram_tensor("yT", [D, C], BF16, kind="ExternalOutput").ap()

    def xsl(t, k):
        """flat slice of the xg tile for (chunk t, k-tile k)"""
        return bass.ds(offs[t] * KD + k * chunks[t], chunks[t])

    with _FastExitTileContext(nc) as tc:
        with tc.tile_pool(name="resident", bufs=1) as res_pool, \
             tc.tile_pool(name="w13", bufs=3) as w13_pool, \
             tc.tile_pool(name="w2", bufs=3) as w2_pool, \
             tc.tile_pool(name="ev", bufs=3) as ev_pool, \
             tc.tile_pool(name="psum", bufs=2, space="PSUM") as ps_pool:

            # ---- PE warm-up: dummy matmuls on a zeroed tile while the ----
            # ---- first xg/w DMAs stream in (no deps beyond the memset) ----
            dmy = res_pool.tile([P, 256], BF16, tag="dmy")
            nc.vector.memset(dmy[:], 0.0)
            dmy_ps = ps_pool.tile([P, 256], F32, tag="y", name="dmy_ps")
            for i in range(NDUM):
                nc.tensor.matmul(dmy_ps[:], dmy[:, :P], dmy[:],
                                 start=True, stop=True)

            # Ring plan: the sync HWDGE ring runs its dma_starts strictly
            # FIFO, so the critical path is exactly [w13_0, xg c0] in need
            # order (the SWDGE path spreads transfers across lanes, which
            # makes the first one finish LAST -- keep it off the critical
            # path). Later w13 pairs stream on gpsimd, pool-slot paced; the
            # combine-scale + output stores are on scalar.
            def w13_tile(h):
                return w13_pool.tile([P, 2 * KD * P], BF16, tag=f"w13_{h % 2}",
                                     bufs=2, name=f"w13t_{h}")

            pre13 = {0: w13_tile(0), 1: w13_tile(1)}
            nc.sync.dma_start(pre13[0][:, :KD * P], w13T_d[0][:, :KD * P])
            xg = res_pool.tile([P, KD * C], BF16, tag="xg")
            csl0 = bass.ds(offs[0] * KD, KD * chunks[0])
            nc.sync.dma_start(xg[:, csl0], xgT_d[:, csl0])
            nc.sync.dma_start(pre13[0][:, KD * P:], w13T_d[0][:, KD * P:])
            nc.sync.dma_start(pre13[1][:, :KD * P], w13T_d[1][:, :KD * P])
            nc.sync.dma_start(pre13[1][:, KD * P:], w13T_d[1][:, KD * P:])
            for t in range(1, nt):
                csl = bass.ds(offs[t] * KD, KD * chunks[t])
                nc.sync.dma_start(xg[:, csl], xgT_d[:, csl])
            act = res_pool.tile([P, KH, C], BF16, tag="act")
            scl = res_pool.tile([P, C], F32, tag="scl")

            # ---- stage A: act[H, C] = silu(w1 @ xgT) * (w3 @ xgT) ----
            # h-blocks in pairs, token-chunk loop outside the pair.
            gate_mm = None      # first ph1 group's last MM: gates the
            scl_loaded = False  # gpsimd prefetch + scl out of the window
            for hp in range(0, HB, 2):
                pair = [h for h in (hp, hp + 1) if h < HB]
                w13ts = []
                for i, h in enumerate(pair):
                    if h in pre13:
                        w13ts.append(pre13[h])
                        continue
                    w13t = w13_tile(h)
                    dd = nc.gpsimd.dma_start(w13t[:], w13T_d[h])
                    if hp == 2 and gate_mm is not None:
                        add_dep_helper(dd.ins, gate_mm.ins, sync=True,
                                       reason="keep prefetch out of "
                                              "critical DMA window")
                    w13ts.append(w13t)
                for t in range(nt):
                    for i, h in enumerate(pair):
                        ph1 = ps_pool.tile([P, chunks[t]], F32, tag="h1",
                                           bufs=3, name=f"ph1_{h}_{t}")
                        for k in range(KD):
                            mm = nc.tensor.matmul(
                                ph1[:], w13ts[i][:, k * P:(k + 1) * P],
                                xg[:, xsl(t, k)],
                                start=(k == 0), stop=(k == KD - 1))
                        if gate_mm is None:
                            gate_mm = mm
                        ph3 = ps_pool.tile([P, chunks[t]], F32, tag="h3",
                                           bufs=3, name=f"ph3_{h}_{t}")
                        for k in range(KD):
                            nc.tensor.matmul(
                                ph3[:], w13ts[i][:, KD * P + k * P:KD * P + (k + 1) * P],
                                xg[:, xsl(t, k)],
                                start=(k == 0), stop=(k == KD - 1))
                        asl = act[:, h, bass.ds(offs[t], chunks[t])]
                        nc.scalar.activation(asl, ph1[:], func=_ACT_FUNC)
                        nc.vector.tensor_mul(asl, asl, ph3[:])
                if not scl_loaded:
                    # combine-weight row, needed only for stage B evictions
                    ds = nc.scalar.dma_start(scl[:], scl_d[:, :])
                    add_dep_helper(ds.ins, gate_mm.ins, sync=True,
                                   reason="scl load after critical window")
                    scl_loaded = True

            # ---- stage B: yT[D, C] = (w2 @ act) * scale ----
            # The very last (d, t) group is split in halves so the final
            # eviction + store trail a 256-col matmul group, not a 512 one.
            for d in range(DB):
                w2t = w2_pool.tile([P, KH * P], BF16, tag="w2")
                nc.sync.dma_start(w2t[:], w2T_d[d])
                for t in range(nt):
                    tsl = bass.ds(offs[t], chunks[t])
                    last = (d == DB - 1 and t == nt - 1)
                    halves = 2 if (last and chunks[t] >= 512) else 1
                    hw = chunks[t] // halves
                    for q in range(halves):
                        qsl = bass.ds(offs[t] + q * hw, hw)
                        py = ps_pool.tile([P, hw], F32, tag="y")
                        for k in range(KH):
                            nc.tensor.matmul(
                                py[:], w2t[:, k * P:(k + 1) * P],
                                act[:, k, qsl],
                                start=(k == 0), stop=(k == KH - 1))
                        ysb = ev_pool.tile([P, hw], BF16, tag="ysb")
                        nc.vector.tensor_mul(ysb[:], py[:], scl[:, qsl])
                        nc.scalar.dma_start(
                            yT_d[d * P:(d + 1) * P, qsl], ysb[:])

    nc.compile()
    return nc


def _route(flat, gate_w):
    """Host replica of the reference router. Returns top-2 expert ids and
    combine weights (top-2 of softmax, renormalized)."""
    logits = flat @ gate_w.T                                   # [N, E] f32
    m = logits.max(axis=1, keepdims=True)
    p = np.exp((logits - m).astype(np.float32))
    probs = p / p.sum(axis=1, keepdims=True)
    idx = np.argsort(-probs, axis=1, kind="stable")[:, :TOPK]  # [N, 2]
    top = np.take_along_axis(probs, idx, axis=1)               # [N, 2]
    wn = top / top.sum(axis=1, keepdims=True)
    return idx, wn


def kernel(x, gate_w, w1, w2, w3):
    global LAST_RESULTS
    x = np.asarray(x, np.float32)
    gate_w = np.asarray(gate_w, np.float32)
    w1 = np.asarray(w1, np.float32)
    w2 = np.asarray(w2, np.float32)
    w3 = np.asarray(w3, np.float32)

    flat = x.reshape(N, D)
    idx, wn = _route(flat, gate_w)

    sels, wsels = [], []
    for e in range(E):
        hit = idx == e                                         # [N, 2]
        sel = np.nonzero(hit.any(axis=1))[0]
        k = hit[sel, 1].astype(np.int64)                       # which top slot
        sels.append(sel)
        wsels.append(wn[sel, k])
    cmax = max(len(s) for s in sels)
    chunks = _chunk_plan(cmax)
    offs = [sum(chunks[:i]) for i in range(len(chunks))]
    C = sum(chunks)

    xbf = flat.astype(NPBF16)                                  # [N, D]
    in_maps = []
    for e in range(E):
        sel = sels[e][:C]                  # tokens beyond C go to _host_ffn
        # chunk-blocked xg: [P, KD*C]; chunk t, k-tile k at flat offset
        # offs[t]*KD + k*chunks[t], holding x[sel_chunk, k*P+p] columns
        xgT = np.zeros((P, KD * C), NPBF16)
        xrows = xbf[sel]                                       # [n_sel, D]
        for t, (o, cs) in enumerate(zip(offs, chunks)):
            seg = xrows[o:o + cs]                              # [<=cs, D]
            n = len(seg)
            if n == 0:
                continue
            # seg.T reshaped to k-tiles: [KD, P, n] -> dest [P, KD, cs]
            blk = np.zeros((KD, P, cs), NPBF16)
            blk[:, :, :n] = seg.T.reshape(KD, P, n)
            xgT[:, o * KD:(o + cs) * KD] = (
                blk.transpose(1, 0, 2).reshape(P, KD * cs))
        scale_b = np.zeros((P, C), np.float32)
        scale_b[:, :len(sel)] = wsels[e][:C][None, :]
        w13 = np.concatenate([_pretile(w1[e], HB), _pretile(w3[e], HB)],
                             axis=2)
        in_maps.append({
            "xgT": xgT,
            "w13T": np.ascontiguousarray(w13),    # [HB, P, 2*KD*P]
            "w2T": _pretile(w2[e], DB),
            "scale_b": scale_b,
        })

    key = tuple(chunks)
    if key not in _program_cache:
        _program_cache[key] = _build_program(chunks)
    nc = _program_cache[key]

    res = run_bass_kernel_spmd(
        nc, in_maps, core_ids=list(range(E)),
        trace=TRACE,
        trace_cores=list(range(E)) if (TRACE and TRACE_ALL_CORES) else None,
    )
    LAST_RESULTS = res

    out = np.zeros((N, D), np.float32)
    for e in range(E):
        sel = sels[e][:C]
        out[sel] += res.results[e]["yT"][:, :len(sel)].T.astype(np.float32)
        over = sels[e][C:]
        if len(over):
            out[over] += _host_ffn(flat[over], w1[e], w2[e], w3[e],
                                   wsels[e][C:])
    return out.reshape(B, T, D)
